# revision 6
# baseline (speedup 1.0000x reference)
"""Trainium2 Bass kernel for nn_Diversity2 (per-row Pearson correlation of
temperature softmaxes, averaged).

Math: for each row r of x1, x2 [N, C]:
    p = softmax(x/T) and Pearson corr is invariant to the per-row positive
    scaling and mean-centering, so with d = exp(x/T) - 1:
      Z'  = sum_c d            (= Z - C, from ACT accum of exp pass)
      S11 = sum_c d1^2, S22 = sum_c d2^2, S12 = sum_c d1*d2
      corr = (S12 - Z1'Z2'/C) / sqrt((S11 - Z1'^2/C)(S22 - Z2'^2/C))
    answer = SCALE * mean_r(corr)

The d-offset (subtracting 1 from e=exp(x/T)~1.0) keeps every accumulated
sum well-conditioned in fp32 (no ~1000-sized terms cancelling).

Sharding: data-parallel over rows, 8192 rows per core, 64 tiles of
[128 rows, 1000 classes] each. Per-core output is a [128,1] column of
partial corr sums; host combines 8*128 partials in fp64.

Engine split per tile (DMA ~2.9us is the roofline):
  ACT: e1=Exp(x1/T)+accum Z1, e2=Exp(x2/T)+accum Z2, Square(e1-1)+accum S11
  DVE: d2=e2-1 (2x tensor_scalar), ttr(d2,d2)+accum S22, stt((e1-1)*d2)+accum S12
"""

import sys

if "/opt/trn_rl_repo" not in sys.path:
    sys.path.insert(0, "/opt/trn_rl_repo")

import numpy as np

T_INV = 1.0 / 20.0
SCALE = 0.3
N_ROWS = 65536
C = 1000
N_CORES = 8
P = 128
ROWS_PER_CORE = N_ROWS // N_CORES  # 8192
N_TILES_FULL = ROWS_PER_CORE // P  # 64

_PROG_CACHE: dict = {}


def build_program(n_tiles: int = N_TILES_FULL, num_devices: int = N_CORES):
    import concourse.tile as tile
    from concourse import bacc, mybir

    f32 = mybir.dt.float32
    AF = mybir.ActivationFunctionType
    OP = mybir.AluOpType

    nc = bacc.Bacc(
        "TRN2", target_bir_lowering=False, debug=False, num_devices=num_devices
    )
    rows = n_tiles * P
    X1 = nc.dram_tensor("x1", [rows, C], f32, kind="ExternalInput").ap()
    X2 = nc.dram_tensor("x2", [rows, C], f32, kind="ExternalInput").ap()
    OUT = nc.dram_tensor("out", [P, 1], f32, kind="ExternalOutput").ap()

    with tile.TileContext(nc) as tc:
        with (
            tc.tile_pool(name="pin", bufs=6) as pin,
            tc.tile_pool(name="pe", bufs=3) as pe,
            tc.tile_pool(name="pd", bufs=3) as pd,
            tc.tile_pool(name="pscr", bufs=2) as pscr,
            tc.tile_pool(name="pstat", bufs=1) as pstat,
        ):
            neg1 = pstat.tile([P, 1], f32, tag="neg1")
            nc.vector.memset(neg1[:], -1.0)

            Z1A = pstat.tile([P, n_tiles], f32, tag="z1a")
            Z2A = pstat.tile([P, n_tiles], f32, tag="z2a")
            S11A = pstat.tile([P, n_tiles], f32, tag="s11a")
            S22A = pstat.tile([P, n_tiles], f32, tag="s22a")
            S12A = pstat.tile([P, n_tiles], f32, tag="s12a")

            for i in range(n_tiles):
                x1t = pin.tile([P, C], f32, tag="x1t")
                nc.sync.dma_start(out=x1t[:], in_=X1[i * P : (i + 1) * P, :])
                x2t = pin.tile([P, C], f32, tag="x2t")
                nc.sync.dma_start(out=x2t[:], in_=X2[i * P : (i + 1) * P, :])

                # e = exp(x/T); accum gives Z = sum(e) = C + Z'
                e1 = pe.tile([P, C], f32, tag="e1")
                nc.scalar.activation(
                    e1[:], x1t[:], AF.Exp, scale=T_INV, accum_out=Z1A[:, i : i + 1]
                )
                e2 = pe.tile([P, C], f32, tag="e2")
                nc.scalar.activation(
                    e2[:], x2t[:], AF.Exp, scale=T_INV, accum_out=Z2A[:, i : i + 1]
                )

                # d2 = e2 - 1  (fp32 tensor_scalar runs 2x mode)
                d2 = pd.tile([P, C], f32, tag="d2")
                nc.vector.tensor_scalar(d2[:], e2[:], -1.0, None, OP.add)

                # S11 = sum((e1-1)^2) on ACT: Square(in + bias)
                scr_a = pscr.tile([P, C], f32, tag="scr_a")
                nc.scalar.activation(
                    scr_a[:],
                    e1[:],
                    AF.Square,
                    bias=neg1[:],
                    accum_out=S11A[:, i : i + 1],
                )

                # S22 = sum((e2-1)*d2) = sum(d2^2), fused on DVE
                # (tensor_tensor_reduce is NOT used: it hard-faults the device)
                scr_d = pscr.tile([P, C], f32, tag="scr_d")
                nc.vector.scalar_tensor_tensor(
                    out=scr_d[:],
                    in0=e2[:],
                    scalar=-1.0,
                    in1=d2[:],
                    op0=OP.add,
                    op1=OP.mult,
                    accum_out=S22A[:, i : i + 1],
                )

                # S12 = sum((e1-1)*d2), fused on DVE
                scr_m = pscr.tile([P, C], f32, tag="scr_m")
                nc.vector.scalar_tensor_tensor(
                    out=scr_m[:],
                    in0=e1[:],
                    scalar=-1.0,
                    in1=d2[:],
                    op0=OP.add,
                    op1=OP.mult,
                    accum_out=S12A[:, i : i + 1],
                )

            # ---- final stage on [P, n_tiles] stat arrays ----
            nt = n_tiles

            def ftile(tag):
                return pstat.tile([P, nt], f32, tag=tag, name=tag)

            z1p = ftile("z1p")
            nc.vector.tensor_scalar(z1p[:], Z1A[:], -float(C), None, OP.add)
            z2p = ftile("z2p")
            nc.vector.tensor_scalar(z2p[:], Z2A[:], -float(C), None, OP.add)

            # num = S12 - z1p*z2p/C
            t0 = ftile("t0")
            nc.vector.tensor_mul(t0[:], z1p[:], z2p[:])
            num = ftile("num")
            nc.vector.scalar_tensor_tensor(
                out=num[:], in0=t0[:], scalar=-1.0 / C, in1=S12A[:],
                op0=OP.mult, op1=OP.add,
            )
            # b = S11 - z1p^2/C ; c = S22 - z2p^2/C
            q1 = ftile("q1")
            nc.vector.tensor_mul(q1[:], z1p[:], z1p[:])
            bv = ftile("bv")
            nc.vector.scalar_tensor_tensor(
                out=bv[:], in0=q1[:], scalar=-1.0 / C, in1=S11A[:],
                op0=OP.mult, op1=OP.add,
            )
            q2 = ftile("q2")
            nc.vector.tensor_mul(q2[:], z2p[:], z2p[:])
            cv = ftile("cv")
            nc.vector.scalar_tensor_tensor(
                out=cv[:], in0=q2[:], scalar=-1.0 / C, in1=S22A[:],
                op0=OP.mult, op1=OP.add,
            )

            bc = ftile("bc")
            nc.vector.tensor_mul(bc[:], bv[:], cv[:])

            # r ~= rsqrt(bc): ACT sqrt (loose table) + DVE reciprocal,
            # then two Newton steps to fp32 accuracy.
            sq = ftile("sq")
            nc.scalar.sqrt(sq[:], bc[:])
            r = ftile("r")
            nc.vector.reciprocal(r[:], sq[:])
            for it in range(2):
                y = ftile(f"y{it}")
                nc.vector.tensor_mul(y[:], r[:], r[:])
                w = ftile(f"w{it}")
                nc.vector.tensor_mul(w[:], bc[:], y[:])
                fcorr = ftile(f"f{it}")
                nc.vector.tensor_scalar(fcorr[:], w[:], -0.5, 1.5, OP.mult, OP.add)
                r2 = ftile(f"r{it}")
                nc.vector.tensor_mul(r2[:], r[:], fcorr[:])
                r = r2

            corr = ftile("corr")
            nc.vector.tensor_mul(corr[:], num[:], r[:])

            csum = pstat.tile([P, 1], f32, tag="csum")
            nc.vector.tensor_reduce(
                csum[:], corr[:], axis=mybir.AxisListType.X, op=OP.add
            )
            nc.sync.dma_start(out=OUT[:], in_=csum[:])

    nc.compile()
    return nc


def _get_program():
    key = "full"
    if key not in _PROG_CACHE:
        _PROG_CACHE[key] = build_program()
    return _PROG_CACHE[key]


def run_sharded(outputs1: np.ndarray, outputs2: np.ndarray, trace: bool = False):
    from concourse.bass_utils import run_bass_kernel_spmd

    nc = _get_program()
    x1 = np.ascontiguousarray(outputs1, dtype=np.float32)
    x2 = np.ascontiguousarray(outputs2, dtype=np.float32)
    in_maps = [
        {
            "x1": x1[i * ROWS_PER_CORE : (i + 1) * ROWS_PER_CORE],
            "x2": x2[i * ROWS_PER_CORE : (i + 1) * ROWS_PER_CORE],
        }
        for i in range(N_CORES)
    ]
    res = run_bass_kernel_spmd(nc, in_maps, list(range(N_CORES)), trace=trace)
    total = 0.0
    for r in res.results:
        total += r["out"].astype(np.float64).sum()
    val = SCALE * total / float(N_ROWS)
    return np.asarray(val, dtype=np.float32), res


def kernel(outputs1, outputs2, targets=None, **_unused):
    val, _ = run_sharded(np.asarray(outputs1), np.asarray(outputs2))
    return val


# revision 8
# speedup vs baseline: 1.0793x; 1.0793x over previous
"""Trainium2 Bass kernel for nn_Diversity2 (per-row Pearson correlation of
temperature softmaxes, averaged).

Math: for each row r of x1, x2 [N, C]:
    p = softmax(x/T) and Pearson corr is invariant to the per-row positive
    scaling and mean-centering, so with d = exp(x/T) - 1:
      Z'  = sum_c d            (= Z - C, from ACT accum of exp pass)
      S11 = sum_c d1^2, S22 = sum_c d2^2, S12 = sum_c d1*d2
      corr = (S12 - Z1'Z2'/C) / sqrt((S11 - Z1'^2/C)(S22 - Z2'^2/C))
    answer = SCALE * mean_r(corr)

The d-offset (subtracting 1 from e=exp(x/T)~1.0) keeps every accumulated
sum well-conditioned in fp32 (no ~1000-sized terms cancelling).

Sharding: data-parallel over rows, 8192 rows per core, 64 tiles of
[128 rows, 1000 classes] each. Per-core output is a [128,1] column of
partial corr sums; host combines 8*128 partials in fp64.

Engine split per tile (DMA ~2.9us is the roofline):
  ACT: e1=Exp(x1/T)+accum Z1, e2=Exp(x2/T)+accum Z2, Square(e1-1)+accum S11
  DVE: d2=e2-1 (2x tensor_scalar), ttr(d2,d2)+accum S22, stt((e1-1)*d2)+accum S12
"""

import sys

if "/opt/trn_rl_repo" not in sys.path:
    sys.path.insert(0, "/opt/trn_rl_repo")

import numpy as np

T_INV = 1.0 / 20.0
SCALE = 0.3
N_ROWS = 65536
C = 1000
N_CORES = 8
P = 128
ROWS_PER_CORE = N_ROWS // N_CORES  # 8192
N_TILES_FULL = ROWS_PER_CORE // P  # 64

_PROG_CACHE: dict = {}


def build_program(n_tiles: int = N_TILES_FULL, num_devices: int = N_CORES):
    import concourse.tile as tile
    from concourse import bacc, mybir

    f32 = mybir.dt.float32
    AF = mybir.ActivationFunctionType
    OP = mybir.AluOpType

    nc = bacc.Bacc(
        "TRN2", target_bir_lowering=False, debug=False, num_devices=num_devices
    )
    rows = n_tiles * P
    X1 = nc.dram_tensor("x1", [rows, C], f32, kind="ExternalInput").ap()
    X2 = nc.dram_tensor("x2", [rows, C], f32, kind="ExternalInput").ap()
    OUT = nc.dram_tensor("out", [P, 1], f32, kind="ExternalOutput").ap()

    with tile.TileContext(nc) as tc:
        with (
            tc.tile_pool(name="pin", bufs=6) as pin,
            tc.tile_pool(name="pe", bufs=3) as pe,
            tc.tile_pool(name="pd", bufs=3) as pd,
            tc.tile_pool(name="pscr", bufs=2) as pscr,
            tc.tile_pool(name="pstat", bufs=1) as pstat,
        ):
            neg1 = pstat.tile([P, 1], f32, tag="neg1")
            nc.vector.memset(neg1[:], -1.0)

            Z1A = pstat.tile([P, n_tiles], f32, tag="z1a")
            Z2A = pstat.tile([P, n_tiles], f32, tag="z2a")
            S11A = pstat.tile([P, n_tiles], f32, tag="s11a")
            S22A = pstat.tile([P, n_tiles], f32, tag="s22a")
            S12A = pstat.tile([P, n_tiles], f32, tag="s12a")

            for i in range(n_tiles):
                x1t = pin.tile([P, C], f32, tag="x1t")
                nc.sync.dma_start(out=x1t[:], in_=X1[i * P : (i + 1) * P, :])
                x2t = pin.tile([P, C], f32, tag="x2t")
                nc.sync.dma_start(out=x2t[:], in_=X2[i * P : (i + 1) * P, :])

                # e = exp(x/T); accum gives Z = sum(e) = C + Z'
                e1 = pe.tile([P, C], f32, tag="e1")
                nc.scalar.activation(
                    e1[:], x1t[:], AF.Exp, scale=T_INV, accum_out=Z1A[:, i : i + 1]
                )
                e2 = pe.tile([P, C], f32, tag="e2")
                nc.scalar.activation(
                    e2[:], x2t[:], AF.Exp, scale=T_INV, accum_out=Z2A[:, i : i + 1]
                )

                # d2 = e2 - 1  (fp32 tensor_scalar runs 2x mode)
                d2 = pd.tile([P, C], f32, tag="d2")
                nc.vector.tensor_scalar(d2[:], e2[:], -1.0, None, OP.add)

                # S11 = sum((e1-1)^2). ACT is the busiest engine (2 exp
                # passes are ACT-only), so every 4th tile computes it on DVE
                # instead as sum((e1-1)*e1) = S11 + Z1' (corrected in the
                # final stage); the rest use ACT Square(in + bias).
                if i % 4 == 0:
                    scr_s = pscr.tile([P, C], f32, tag="scr_s")
                    nc.vector.scalar_tensor_tensor(
                        out=scr_s[:],
                        in0=e1[:],
                        scalar=-1.0,
                        in1=e1[:],
                        op0=OP.add,
                        op1=OP.mult,
                        accum_out=S11A[:, i : i + 1],
                    )
                else:
                    scr_a = pscr.tile([P, C], f32, tag="scr_a")
                    nc.scalar.activation(
                        scr_a[:],
                        e1[:],
                        AF.Square,
                        bias=neg1[:],
                        accum_out=S11A[:, i : i + 1],
                    )

                # S22 = sum((e2-1)*d2) = sum(d2^2), fused on DVE
                # (tensor_tensor_reduce is NOT used: it hard-faults the device)
                scr_d = pscr.tile([P, C], f32, tag="scr_d")
                nc.vector.scalar_tensor_tensor(
                    out=scr_d[:],
                    in0=e2[:],
                    scalar=-1.0,
                    in1=d2[:],
                    op0=OP.add,
                    op1=OP.mult,
                    accum_out=S22A[:, i : i + 1],
                )

                # S12 = sum((e1-1)*d2), fused on DVE
                scr_m = pscr.tile([P, C], f32, tag="scr_m")
                nc.vector.scalar_tensor_tensor(
                    out=scr_m[:],
                    in0=e1[:],
                    scalar=-1.0,
                    in1=d2[:],
                    op0=OP.add,
                    op1=OP.mult,
                    accum_out=S12A[:, i : i + 1],
                )

            # ---- final stage on [P, n_tiles] stat arrays ----
            nt = n_tiles

            def ftile(tag):
                return pstat.tile([P, nt], f32, tag=tag, name=tag)

            z1p = ftile("z1p")
            nc.vector.tensor_scalar(z1p[:], Z1A[:], -float(C), None, OP.add)
            z2p = ftile("z2p")
            nc.vector.tensor_scalar(z2p[:], Z2A[:], -float(C), None, OP.add)

            # correct the DVE-computed S11 columns: stored sum was
            # sum((e1-1)*e1) = S11 + Z1'
            nc.vector.tensor_sub(
                S11A[:, 0 : n_tiles : 4], S11A[:, 0 : n_tiles : 4],
                z1p[:, 0 : n_tiles : 4],
            )

            # num = S12 - z1p*z2p/C
            t0 = ftile("t0")
            nc.vector.tensor_mul(t0[:], z1p[:], z2p[:])
            num = ftile("num")
            nc.vector.scalar_tensor_tensor(
                out=num[:], in0=t0[:], scalar=-1.0 / C, in1=S12A[:],
                op0=OP.mult, op1=OP.add,
            )
            # b = S11 - z1p^2/C ; c = S22 - z2p^2/C
            q1 = ftile("q1")
            nc.vector.tensor_mul(q1[:], z1p[:], z1p[:])
            bv = ftile("bv")
            nc.vector.scalar_tensor_tensor(
                out=bv[:], in0=q1[:], scalar=-1.0 / C, in1=S11A[:],
                op0=OP.mult, op1=OP.add,
            )
            q2 = ftile("q2")
            nc.vector.tensor_mul(q2[:], z2p[:], z2p[:])
            cv = ftile("cv")
            nc.vector.scalar_tensor_tensor(
                out=cv[:], in0=q2[:], scalar=-1.0 / C, in1=S22A[:],
                op0=OP.mult, op1=OP.add,
            )

            bc = ftile("bc")
            nc.vector.tensor_mul(bc[:], bv[:], cv[:])

            # r ~= rsqrt(bc): ACT sqrt (loose table) + DVE reciprocal,
            # then two Newton steps to fp32 accuracy.
            sq = ftile("sq")
            nc.scalar.sqrt(sq[:], bc[:])
            r = ftile("r")
            nc.vector.reciprocal(r[:], sq[:])
            for it in range(2):
                y = ftile(f"y{it}")
                nc.vector.tensor_mul(y[:], r[:], r[:])
                w = ftile(f"w{it}")
                nc.vector.tensor_mul(w[:], bc[:], y[:])
                fcorr = ftile(f"f{it}")
                nc.vector.tensor_scalar(fcorr[:], w[:], -0.5, 1.5, OP.mult, OP.add)
                r2 = ftile(f"r{it}")
                nc.vector.tensor_mul(r2[:], r[:], fcorr[:])
                r = r2

            corr = ftile("corr")
            nc.vector.tensor_mul(corr[:], num[:], r[:])

            csum = pstat.tile([P, 1], f32, tag="csum")
            nc.vector.tensor_reduce(
                csum[:], corr[:], axis=mybir.AxisListType.X, op=OP.add
            )
            nc.sync.dma_start(out=OUT[:], in_=csum[:])

    nc.compile()
    return nc


def _get_program():
    key = "full"
    if key not in _PROG_CACHE:
        _PROG_CACHE[key] = build_program()
    return _PROG_CACHE[key]


def run_sharded(outputs1: np.ndarray, outputs2: np.ndarray, trace: bool = False):
    from concourse.bass_utils import run_bass_kernel_spmd

    nc = _get_program()
    x1 = np.ascontiguousarray(outputs1, dtype=np.float32)
    x2 = np.ascontiguousarray(outputs2, dtype=np.float32)
    in_maps = [
        {
            "x1": x1[i * ROWS_PER_CORE : (i + 1) * ROWS_PER_CORE],
            "x2": x2[i * ROWS_PER_CORE : (i + 1) * ROWS_PER_CORE],
        }
        for i in range(N_CORES)
    ]
    res = run_bass_kernel_spmd(nc, in_maps, list(range(N_CORES)), trace=trace)
    total = 0.0
    for r in res.results:
        total += r["out"].astype(np.float64).sum()
    val = SCALE * total / float(N_ROWS)
    return np.asarray(val, dtype=np.float32), res


def kernel(outputs1, outputs2, targets=None, **_unused):
    val, _ = run_sharded(np.asarray(outputs1), np.asarray(outputs2))
    return val
